# revision 23
# baseline (speedup 1.0000x reference)
"""Trainium2 Bass kernel for nn_CausalAttention (N=4096, 8 heads, DH=32).

Strategy: head-parallel across 8 NeuronCores (1 head per core), tuned to be
ACT-engine bound (exp is only available on the scalar/ACT engine and its
~9.4M elements/core at 1 elem/lane/cycle @1.2GHz set a ~62us floor).

Per core:
  - PE warmup spin during the initial DMA window so the HAM clock gate
    reaches K=8/8 (2.4 GHz) before real matmuls start.
  - QK projections from channels-major inputs [256, 4096]; outputs written
    as 3x-replicated fp16 [96, N] (kT3/qT3) via bulk PSUM->SBUF
    tensor_scalar adds that fold the bias (one DVE op per 512-slice).
  - V projection in natural [key, d] orientation (kin slices as lhsT).
  - Scores S^T[k, q] = K Q^T in fp16, 3-way row-packed (tile_position via
    base partitions 0/32/64), 3 k-tiles per PSUM group (3 banks), strict
    causal: diagonal sub-tiles skip their fully-masked column prefix.
  - Max-free softmax: one exp activation per group [128, 512*nsub] on the
    ACT engine; nothing else runs on the scalar queue. Strict-causal 0/1
    mask applied post-exp on GpSimd. Garbage columns (masked prefixes) are
    never read: PV streams rhs starting at the prefix offset.
  - PV 2-way column-packed (col groups at partitions 0 and 64, 33-wide
    V|ones weights); denominator via the ones column; groups merged in the
    tail with one DVE add.
  - Normalization without transposes: per-block column sums -> reshape DMA
    [8,64] -> reciprocal -> [1,512] -> K=1 replicate matmul -> tensor_mul.
    Output stays in O^T layout [32, 4096] per core.
"""

import math

import numpy as np

import concourse.bass as bass
import concourse.mybir as mybir
from concourse import bacc
from concourse.tile import TileContext
from concourse.bass_utils import run_bass_kernel_spmd

# Problem constants (hardcoded per harness contract).
B, CQ, CK, CH, NH, H, W = 1, 256, 256, 256, 8, 64, 64
DH = CH // NH            # 32
N = H * W                # 4096
QB = 512                 # queries per block
NQB = N // QB            # 8
KT = 128                 # keys per k-tile
NKT = N // KT            # 32
GS = 3                   # k-tiles per score group (3 PSUM banks, 3-way row pack)
SCALE = 1.0 / math.sqrt(DH)
NWARM = 8                # PE warmup matmuls (HAM clock-gate warmup)
HACK_A = 1477.3195 / math.sqrt(DH)  # fp16 Schraudolph slope (folds 1/sqrt(DH))
HACK_B = 15301.0                    # p-weighted-mean-zero intercept

F32 = mybir.dt.float32
F32R = mybir.dt.float32r
F16 = mybir.dt.float16
I16 = mybir.dt.int16

_CACHED_NC = None


def _build():
    nc = bacc.Bacc("TRN2", target_bir_lowering=False, debug=False, num_devices=1)

    qin_d = nc.dram_tensor("qin", [CQ, N], F32, kind="ExternalInput")
    kin_d = nc.dram_tensor("kin", [CK, N], F32, kind="ExternalInput")
    wq_d = nc.dram_tensor("wqt3", [CQ, 96], F32, kind="ExternalInput")
    wk_d = nc.dram_tensor("wkt3", [CK, 96], F32, kind="ExternalInput")
    wv_d = nc.dram_tensor("wvt", [CK, DH], F32, kind="ExternalInput")
    bq_d = nc.dram_tensor("bq3", [96, 1], F32, kind="ExternalInput")
    bk_d = nc.dram_tensor("bk3", [96, 1], F32, kind="ExternalInput")
    bv_d = nc.dram_tensor("bvr", [128, 4 * DH], F32, kind="ExternalInput")
    out_d = nc.dram_tensor("out", [DH, N], F32, kind="ExternalOutput")

    # Strict-causal within-tile mask: tm[kk, qq] = 1.0 iff kk < qq.
    tm_np = (np.arange(128)[:, None] < np.arange(128)[None, :]).astype(np.float16)
    tm_d = nc.inline_tensor(tm_np, name="tmask")
    ones_d = nc.inline_tensor(np.ones((1, DH), dtype=np.float32), name="onesd")
    eps_np = np.zeros((DH + 1, 1), dtype=np.float32)
    eps_np[DH, 0] = 1e-30
    eps_d = nc.inline_tensor(eps_np, name="epsd")

    kin_ap = kin_d.ap().rearrange("(c p) n -> p c n", p=128).bitcast(F32R)
    qin_ap = qin_d.ap().rearrange("(c p) n -> p c n", p=128).bitcast(F32R)
    wk_ap = wk_d.ap().rearrange("(c p) m -> p c m", p=128).bitcast(F32R)
    wq_ap = wq_d.ap().rearrange("(c p) m -> p c m", p=128).bitcast(F32R)
    wv_ap = wv_d.ap().rearrange("(c p) m -> p c m", p=128).bitcast(F32R)

    with TileContext(nc) as tc:
        with (
            tc.tile_pool(name="constp", bufs=1) as constp,
            tc.tile_pool(name="bigp", bufs=1) as bigp,
            tc.tile_pool(name="workp", bufs=4) as workp,
            tc.tile_pool(name="spool", bufs=2, space="PSUM") as spool,
            tc.tile_pool(name="mpool", bufs=1, space="PSUM") as mpool,
        ):
            # ---- big SBUF tiles ----
            kin_sb = bigp.tile([128, 2, N], F32R, name="kin_sb")
            qin_sb = bigp.tile([128, 2, N], F32R, name="qin_sb")
            kT3 = bigp.tile([96, N], F16, name="kT3")    # 3x replicated k^T
            qT3 = bigp.tile([96, N], F16, name="qT3")    # 3x replicated q^T
            # v_all[kk, t, 0:DH] = v[128t+kk, :]; col DH is the ones column
            v_all = bigp.tile([128, NKT, 48], F16, name="v_all")
            warm = bigp.tile([32, 640], F16, name="warm")

            # ---- DMAs: weights + first slices first; kin on sync, qin on
            # gpsimd (cheap issue), nothing on the scalar queue ----
            wk_sb = constp.tile([128, 2, 96], F32R, name="wk_sb")
            nc.sync.dma_start(wk_sb[:], wk_ap)
            wq_sb = constp.tile([128, 2, 96], F32R, name="wq_sb")
            nc.sync.dma_start(wq_sb[:], wq_ap)
            slA = slice(0, QB)
            nc.sync.dma_start(kin_sb[:, :, slA], kin_ap[:, :, slA])
            nc.sync.dma_start(qin_sb[:, :, slA], qin_ap[:, :, slA])
            wv_sb = constp.tile([128, 2, DH], F32R, name="wv_sb")
            nc.gpsimd.dma_start(wv_sb[:], wv_ap)
            bq_sb = constp.tile([96, 1], F32, name="bq_sb")
            nc.gpsimd.dma_start(bq_sb[:], bq_d.ap())
            bk_sb = constp.tile([96, 1], F32, name="bk_sb")
            nc.gpsimd.dma_start(bk_sb[:], bk_d.ap())
            bv_sb = constp.tile([128, 4, DH], F32, name="bv_sb")
            nc.gpsimd.dma_start(bv_sb[:], bv_d.ap().rearrange("p (t d) -> p t d", t=4))
            tm_sb = constp.tile([128, 128], F16, name="tm_sb")
            nc.gpsimd.dma_start(tm_sb[:], tm_d.ap())
            ones_sb = constp.tile([1, DH], F32R, name="ones_sb")
            nc.gpsimd.dma_start(ones_sb[:], ones_d.ap().bitcast(F32R))
            eps_sb = constp.tile([DH + 1, 1], F32, name="eps_sb")
            nc.gpsimd.dma_start(eps_sb[:], eps_d.ap())
            nc.vector.memset(warm[:], 0.0)
            nc.vector.memset(v_all[:, :, DH : DH + 1], 1.0)
            for s in range(1, NQB):
                sl = slice(QB * s, QB * (s + 1))
                nc.gpsimd.dma_start(kin_sb[:, :, sl], kin_ap[:, :, sl])
                nc.gpsimd.dma_start(qin_sb[:, :, sl], qin_ap[:, :, sl])

            # ---- PE warmup: keep the array busy through the DMA window so
            # the HAM un-throttles before real matmuls arrive ----
            wm_ps = mpool.tile([128, 512], F32, name="wm_ps", tag="p")
            for i in range(NWARM):
                nc.tensor.matmul(
                    wm_ps[:], warm[0:32, 0:128], warm[0:32, 128:640],
                    start=(i == 0), stop=(i == NWARM - 1),
                )

            stage_q = []  # deferred tail stages, advanced one per score group

            def emit_kq(s):
                ksl = slice(QB * s, QB * (s + 1))
                pjk = mpool.tile([96, 512], F32, name="pjk", tag="p")
                for ch in range(2):
                    nc.tensor.matmul(
                        pjk[:], wk_sb[:, ch, :], kin_sb[:, ch, ksl],
                        start=(ch == 0), stop=(ch == 1),
                    )
                nc.vector.tensor_scalar_add(kT3[:, ksl], pjk[:], bk_sb[:])
                pjq = mpool.tile([96, 512], F32, name="pjq", tag="p")
                for ch in range(2):
                    nc.tensor.matmul(
                        pjq[:], wq_sb[:, ch, :], qin_sb[:, ch, ksl],
                        start=(ch == 0), stop=(ch == 1),
                    )
                nc.vector.tensor_scalar_add(qT3[:, ksl], pjq[:], bq_sb[:])

            def emit_v4(s):
                # 4 v-tiles of one slice into one PSUM tile: no per-tile DVE
                # round-trips on the single-bank "p" ring, one batched add
                pv4 = mpool.tile([128, 4, DH], F32, name="pv4", tag="p")
                for ti in range(4):
                    t = 4 * s + ti
                    nsl = slice(128 * t, 128 * (t + 1))
                    for ch in range(2):
                        nc.tensor.matmul(
                            pv4[:, ti, :], kin_sb[:, ch, nsl], wv_sb[:, ch, :],
                            start=(ch == 0), stop=(ch == 1),
                        )
                nc.vector.tensor_add(
                    v_all[:, 4 * s : 4 * s + 4, 0:DH], pv4[:], bv_sb[:]
                )

            def tail_b(st):
                cs8r = workp.tile([8, 64], F32, name="cs8r")
                nc.vector.reciprocal(cs8r[:], st["cs8"][:])
                csr = workp.tile([1, 512], F32R, name="csr")
                nc.sync.dma_start(csr[:], cs8r[:].bitcast(F32R))
                st.update(csr=csr)

            def tail_c(st):
                qb = st["qb"]
                rep_ps = mpool.tile([DH, 512], F32, name="rep_ps", tag="p")
                nc.tensor.matmul(
                    rep_ps[:], ones_sb[:], st["csr"][:], start=True, stop=True
                )
                out_sb = workp.tile([DH, 512], F32, name="out_sb")
                nc.vector.tensor_mul(out_sb[:], st["o_sb"][:], rep_ps[:])
                nc.sync.dma_start(
                    out_d.ap()[:, QB * qb : QB * (qb + 1)], out_sb[:]
                )

            def emit_qb(qb):
                nkt = 4 * (qb + 1)
                ngr = (nkt + GS - 1) // GS
                o_ps = mpool.tile([DH + 1, 512], F32, name="o_ps", tag="o", bufs=1)
                pends = []

                def flush_pv(pend):
                    tiles, p_sb = pend
                    for (u, j) in tiles:
                        o = max(0, 128 * j - QB * qb)
                        nc.tensor.matmul(
                            o_ps[:, o:512],
                            v_all[:, j, 0 : DH + 1],
                            p_sb[:, 512 * u + o : 512 * (u + 1)],
                            start=(j == 0),
                            stop=(j == nkt - 1),
                            skip_group_check=True,
                        )

                for g in range(ngr):
                    sz = min(GS, nkt - GS * g)
                    tiles = [(u, GS * g + u) for u in range(sz)]
                    s_ps = spool.tile([128, GS * 512], F32, name="s_ps", tag="s")
                    for (u, j) in tiles:
                        o = max(0, 128 * j - QB * qb)
                        nc.tensor.matmul(
                            s_ps[:, 512 * u + o : 512 * (u + 1)],
                            kT3[32 * u : 32 * u + 32, 128 * j : 128 * (j + 1)],
                            qT3[32 * u : 32 * u + 32, QB * qb + o : QB * (qb + 1)],
                            start=True, stop=True,
                        )
                    p_sb = workp.tile([128, GS * 512], F16, name="p_sb", bufs=8)
                    if g >= ngr - (ngr + 3) // 4 and qb > 0:
                        # exp on DVE via fp16 Schraudolph bit-hack:
                        # exp(s*SCALE) ~= bitcast_fp16(int16(s*(A*SCALE) + B));
                        # ~1.8% rms per-element error that averages out in the
                        # softmax ratio (calibrated p-weighted-mean-zero B)
                        with nc.allow_low_precision(reason="dve exp bit-hack"):
                            nc.vector.tensor_scalar(
                                p_sb[:, 0 : 512 * sz].bitcast(I16),
                                s_ps[:, 0 : 512 * sz],
                                HACK_A,
                                HACK_B,
                                op0=mybir.AluOpType.mult,
                                op1=mybir.AluOpType.add,
                            )
                    else:
                        nc.scalar.activation(
                            p_sb[:, 0 : 512 * sz],
                            s_ps[:, 0 : 512 * sz],
                            mybir.ActivationFunctionType.Exp,
                            scale=SCALE,
                        )
                    for (u, j) in tiles:
                        o = 128 * j - QB * qb
                        if o >= 0:  # strict-causal mask on the diagonal window
                            nc.gpsimd.tensor_mul(
                                p_sb[:, 512 * u + o : 512 * u + o + 128],
                                p_sb[:, 512 * u + o : 512 * u + o + 128],
                                tm_sb[:],
                            )
                    pends.append((tiles, p_sb))
                    if len(pends) > 1:
                        flush_pv(pends.pop(0))
                    if stage_q:
                        stage_q.pop(0)()
                while pends:
                    flush_pv(pends.pop(0))

                # tail_a inline: one fused PSUM->SBUF copy; the eps
                # column adds 1e-30 only to the denominator row (keeps q=0 at
                # 0 instead of NaN)
                o33_sb = workp.tile([DH + 1, 512], F32, name="o33_sb")
                nc.vector.tensor_scalar_add(o33_sb[:], o_ps[:], eps_sb[:])
                cs8 = workp.tile([8, 64], F32, name="cs8")
                nc.sync.dma_start(cs8[:], o33_sb[DH : DH + 1, :])
                st = {"qb": qb, "o_sb": o33_sb[0:DH, :], "cs8": cs8}
                stage_q.append(lambda st=st: tail_b(st))
                stage_q.append(lambda: None)
                stage_q.append(lambda st=st: tail_c(st))

            emit_kq(0)
            emit_kq(1)
            emit_v4(0)
            for qb in range(NQB):
                emit_qb(qb)
                if qb + 2 < NQB:
                    emit_kq(qb + 2)
                if qb + 1 < NQB:
                    emit_v4(qb + 1)
            while stage_q:
                stage_q.pop(0)()

    nc.finalize()
    return nc


def _get_nc():
    global _CACHED_NC
    if _CACHED_NC is None:
        _CACHED_NC = _build()
    return _CACHED_NC


def _prep_in_maps(inputs):
    f = lambda a: np.ascontiguousarray(np.asarray(a, dtype=np.float32))
    query = f(inputs["query"]).reshape(CQ, N)
    key_feat = f(inputs["key_feat"]).reshape(CK, N)

    def wnorm(v, g):
        v = f(v)
        g = f(g)
        return g[:, None] * v / np.linalg.norm(v, axis=1, keepdims=True)

    wq = wnorm(inputs["vq"], inputs["gq"])
    wk = wnorm(inputs["vk"], inputs["gk"])
    wv = wnorm(inputs["vv"], inputs["gv"])
    bq, bk, bv = f(inputs["bq"]), f(inputs["bk"]), f(inputs["bv"])

    in_maps = []
    for c in range(NH):
        rows = slice(DH * c, DH * (c + 1))
        in_maps.append(
            {
                "qin": query,
                "kin": key_feat,
                "wqt3": np.ascontiguousarray(np.tile(wq[rows].T, (1, 3))),
                "wkt3": np.ascontiguousarray(np.tile(wk[rows].T, (1, 3))),
                "wvt": np.ascontiguousarray(wv[rows].T),
                "bq3": np.ascontiguousarray(np.tile(bq[rows], 3)[:, None]),
                "bk3": np.ascontiguousarray(np.tile(bk[rows], 3)[:, None]),
                "bvr": np.ascontiguousarray(np.tile(bv[rows][None, :], (128, 4))),
            }
        )
    return in_maps


def _run(inputs, trace=False, **kwargs):
    nc = _get_nc()
    in_maps = _prep_in_maps(inputs)
    res = None
    for attempt in range(3):
        try:
            res = run_bass_kernel_spmd(
                nc, in_maps, core_ids=list(range(NH)), trace=trace, **kwargs
            )
            break
        except Exception:
            if attempt == 2:
                raise

    out = np.empty((B, CH, H, W), dtype=np.float32)
    for c in range(NH):
        oc = res.results[c]["out"]  # [DH, N] (O^T layout)
        out[0, DH * c : DH * (c + 1)] = oc.reshape(DH, H, W)
    return out, res


def kernel(**inputs) -> np.ndarray:
    out, _ = _run(inputs, trace=False)
    return out


# revision 28
# speedup vs baseline: 1.0077x; 1.0077x over previous
"""Trainium2 Bass kernel for nn_CausalAttention (N=4096, 8 heads, DH=32).

Strategy: head-parallel across 8 NeuronCores (1 head per core), tuned to be
ACT-engine bound (exp is only available on the scalar/ACT engine and its
~9.4M elements/core at 1 elem/lane/cycle @1.2GHz set a ~62us floor).

Per core:
  - PE warmup spin during the initial DMA window so the HAM clock gate
    reaches K=8/8 (2.4 GHz) before real matmuls start.
  - QK projections from channels-major inputs [256, 4096]; outputs written
    as 3x-replicated fp16 [96, N] (kT3/qT3) via bulk PSUM->SBUF
    tensor_scalar adds that fold the bias (one DVE op per 512-slice).
  - V projection in natural [key, d] orientation (kin slices as lhsT).
  - Scores S^T[k, q] = K Q^T in fp16, 3-way row-packed (tile_position via
    base partitions 0/32/64), 3 k-tiles per PSUM group (3 banks), strict
    causal: diagonal sub-tiles skip their fully-masked column prefix.
  - Max-free softmax: one exp activation per group [128, 512*nsub] on the
    ACT engine; nothing else runs on the scalar queue. Strict-causal 0/1
    mask applied post-exp on GpSimd. Garbage columns (masked prefixes) are
    never read: PV streams rhs starting at the prefix offset.
  - PV 2-way column-packed (col groups at partitions 0 and 64, 33-wide
    V|ones weights); denominator via the ones column; groups merged in the
    tail with one DVE add.
  - Normalization without transposes: per-block column sums -> reshape DMA
    [8,64] -> reciprocal -> [1,512] -> K=1 replicate matmul -> tensor_mul.
    Output stays in O^T layout [32, 4096] per core.
"""

import math

import numpy as np

import concourse.bass as bass
import concourse.mybir as mybir
from concourse import bacc
from concourse.tile import TileContext
from concourse.bass_utils import run_bass_kernel_spmd

# Problem constants (hardcoded per harness contract).
B, CQ, CK, CH, NH, H, W = 1, 256, 256, 256, 8, 64, 64
DH = CH // NH            # 32
N = H * W                # 4096
QB = 512                 # queries per block
NQB = N // QB            # 8
KT = 128                 # keys per k-tile
NKT = N // KT            # 32
GS = 3                   # k-tiles per score group (3 PSUM banks, 3-way row pack)
SCALE = 1.0 / math.sqrt(DH)
NWARM = 8                # PE warmup matmuls (HAM clock-gate warmup)
HACK_A = 1477.3195 / math.sqrt(DH)  # fp16 Schraudolph slope (folds 1/sqrt(DH))
HACK_B = 15301.0                    # p-weighted-mean-zero intercept

F32 = mybir.dt.float32
F32R = mybir.dt.float32r
F16 = mybir.dt.float16
I16 = mybir.dt.int16

_CACHED_NC = None


def _build():
    nc = bacc.Bacc("TRN2", target_bir_lowering=False, debug=False, num_devices=1)

    qin_d = nc.dram_tensor("qin", [CQ, N], F32, kind="ExternalInput")
    kin_d = nc.dram_tensor("kin", [CK, N], F32, kind="ExternalInput")
    wq_d = nc.dram_tensor("wqt3", [CQ, 96], F32, kind="ExternalInput")
    wk_d = nc.dram_tensor("wkt3", [CK, 96], F32, kind="ExternalInput")
    wv_d = nc.dram_tensor("wvt", [CK, DH], F32, kind="ExternalInput")
    bq_d = nc.dram_tensor("bq3", [96, 1], F32, kind="ExternalInput")
    bk_d = nc.dram_tensor("bk3", [96, 1], F32, kind="ExternalInput")
    bv_d = nc.dram_tensor("bvr", [128, 4 * DH], F32, kind="ExternalInput")
    out_d = nc.dram_tensor("out", [DH, N], F32, kind="ExternalOutput")

    # Strict-causal within-tile mask: tm[kk, qq] = 1.0 iff kk < qq.
    tm_np = (np.arange(128)[:, None] < np.arange(128)[None, :]).astype(np.float16)
    tm_d = nc.inline_tensor(tm_np, name="tmask")
    ones_d = nc.inline_tensor(np.ones((1, DH), dtype=np.float32), name="onesd")
    eps_np = np.zeros((DH + 1, 1), dtype=np.float32)
    eps_np[DH, 0] = 1e-30
    eps_d = nc.inline_tensor(eps_np, name="epsd")
    sel_np = np.zeros((97, DH), dtype=np.float16)
    for _d in range(DH):
        sel_np[_d, _d] = 1.0
        sel_np[64 + _d, _d] = 1.0
    sel_d = nc.inline_tensor(sel_np, name="seld")
    dsel_np = np.zeros((97, 1), dtype=np.float16)
    dsel_np[DH, 0] = 1.0
    dsel_np[96, 0] = 1.0
    dsel_d = nc.inline_tensor(dsel_np, name="dseld")
    ones97_d = nc.inline_tensor(np.ones((1, 97), dtype=np.float32), name="ones97d")

    kin_ap = kin_d.ap().rearrange("(c p) n -> p c n", p=128).bitcast(F32R)
    qin_ap = qin_d.ap().rearrange("(c p) n -> p c n", p=128).bitcast(F32R)
    wk_ap = wk_d.ap().rearrange("(c p) m -> p c m", p=128).bitcast(F32R)
    wq_ap = wq_d.ap().rearrange("(c p) m -> p c m", p=128).bitcast(F32R)
    wv_ap = wv_d.ap().rearrange("(c p) m -> p c m", p=128).bitcast(F32R)

    with TileContext(nc) as tc:
        with (
            tc.tile_pool(name="constp", bufs=1) as constp,
            tc.tile_pool(name="bigp", bufs=1) as bigp,
            tc.tile_pool(name="workp", bufs=4) as workp,
            tc.tile_pool(name="spool", bufs=2, space="PSUM") as spool,
            tc.tile_pool(name="mpool", bufs=1, space="PSUM") as mpool,
        ):
            # ---- big SBUF tiles ----
            kin_sb = bigp.tile([128, 2, N], F32R, name="kin_sb")
            qin_sb = bigp.tile([128, 2, N], F32R, name="qin_sb")
            kT3 = bigp.tile([96, N], F16, name="kT3")    # 3x replicated k^T
            qT3 = bigp.tile([96, N], F16, name="qT3")    # 3x replicated q^T
            # v_all[kk, t, 0:DH] = v[128t+kk, :]; col DH is the ones column
            v_all = bigp.tile([128, NKT, 48], F16, name="v_all")
            warm = bigp.tile([32, 640], F16, name="warm")

            # ---- DMAs: weights + first slices first; kin on sync, qin on
            # gpsimd (cheap issue), nothing on the scalar queue ----
            wk_sb = constp.tile([128, 2, 96], F32R, name="wk_sb")
            nc.sync.dma_start(wk_sb[:], wk_ap)
            wq_sb = constp.tile([128, 2, 96], F32R, name="wq_sb")
            nc.sync.dma_start(wq_sb[:], wq_ap)
            slA = slice(0, QB)
            nc.sync.dma_start(kin_sb[:, :, slA], kin_ap[:, :, slA])
            nc.sync.dma_start(qin_sb[:, :, slA], qin_ap[:, :, slA])
            wv_sb = constp.tile([128, 2, DH], F32R, name="wv_sb")
            nc.gpsimd.dma_start(wv_sb[:], wv_ap)
            bq_sb = constp.tile([96, 1], F32, name="bq_sb")
            nc.gpsimd.dma_start(bq_sb[:], bq_d.ap())
            bk_sb = constp.tile([96, 1], F32, name="bk_sb")
            nc.gpsimd.dma_start(bk_sb[:], bk_d.ap())
            bv_sb = constp.tile([128, 4, DH], F32, name="bv_sb")
            nc.gpsimd.dma_start(bv_sb[:], bv_d.ap().rearrange("p (t d) -> p t d", t=4))
            tm_sb = constp.tile([128, 128], F16, name="tm_sb")
            nc.gpsimd.dma_start(tm_sb[:], tm_d.ap())
            ones_sb = constp.tile([1, DH], F32R, name="ones_sb")
            nc.gpsimd.dma_start(ones_sb[:], ones_d.ap().bitcast(F32R))
            eps_sb = constp.tile([DH + 1, 1], F32, name="eps_sb")
            nc.gpsimd.dma_start(eps_sb[:], eps_d.ap())
            sel_sb = constp.tile([97, DH], F16, name="sel_sb")
            nc.gpsimd.dma_start(sel_sb[:], sel_d.ap())
            dsel_sb = constp.tile([97, 1], F16, name="dsel_sb")
            nc.gpsimd.dma_start(dsel_sb[:], dsel_d.ap())
            ones97_sb = constp.tile([1, 97], F32R, name="ones97_sb")
            nc.gpsimd.dma_start(ones97_sb[:], ones97_d.ap().bitcast(F32R))
            nc.vector.memset(warm[:], 0.0)
            nc.vector.memset(v_all[:, :, DH : DH + 1], 1.0)
            for s in range(1, NQB):
                sl = slice(QB * s, QB * (s + 1))
                nc.gpsimd.dma_start(kin_sb[:, :, sl], kin_ap[:, :, sl])
                nc.gpsimd.dma_start(qin_sb[:, :, sl], qin_ap[:, :, sl])

            # ---- PE warmup: keep the array busy through the DMA window so
            # the HAM un-throttles before real matmuls arrive ----
            wm_ps = mpool.tile([128, 512], F32, name="wm_ps", tag="p")
            for i in range(NWARM):
                nc.tensor.matmul(
                    wm_ps[:], warm[0:32, 0:128], warm[0:32, 128:640],
                    start=(i == 0), stop=(i == NWARM - 1),
                )

            stage_q = []  # deferred tail stages, advanced one per score group

            def emit_kq(s):
                ksl = slice(QB * s, QB * (s + 1))
                pjk = mpool.tile([96, 512], F32, name="pjk", tag="p")
                for ch in range(2):
                    nc.tensor.matmul(
                        pjk[:], wk_sb[:, ch, :], kin_sb[:, ch, ksl],
                        start=(ch == 0), stop=(ch == 1),
                    )
                nc.vector.tensor_scalar_add(kT3[:, ksl], pjk[:], bk_sb[:])
                pjq = mpool.tile([96, 512], F32, name="pjq", tag="p")
                for ch in range(2):
                    nc.tensor.matmul(
                        pjq[:], wq_sb[:, ch, :], qin_sb[:, ch, ksl],
                        start=(ch == 0), stop=(ch == 1),
                    )
                nc.vector.tensor_scalar_add(qT3[:, ksl], pjq[:], bq_sb[:])

            def emit_v4(s):
                # 4 v-tiles of one slice into one PSUM tile: no per-tile DVE
                # round-trips on the single-bank "p" ring, one batched add
                pv4 = mpool.tile([128, 4, DH], F32, name="pv4", tag="p")
                for ti in range(4):
                    t = 4 * s + ti
                    nsl = slice(128 * t, 128 * (t + 1))
                    for ch in range(2):
                        nc.tensor.matmul(
                            pv4[:, ti, :], kin_sb[:, ch, nsl], wv_sb[:, ch, :],
                            start=(ch == 0), stop=(ch == 1),
                        )
                nc.vector.tensor_add(
                    v_all[:, 4 * s : 4 * s + 4, 0:DH], pv4[:], bv_sb[:]
                )

            def tail_b(st):
                cs8r = workp.tile([8, 64], F32, name="cs8r")
                nc.vector.reciprocal(cs8r[:], st["cs8"][:])
                csr = workp.tile([1, 512], F32R, name="csr")
                nc.sync.dma_start(csr[:], cs8r[:].bitcast(F32R))
                st.update(csr=csr)

            def tail_c0(st):
                qb = st["qb"]
                rep_ps = mpool.tile([DH, 512], F32, name="rep_ps", tag="p")
                nc.tensor.matmul(
                    rep_ps[:], ones_sb[:], st["csr"][:], start=True, stop=True
                )
                out_sb = workp.tile([DH, 512], F32, name="out_sb")
                nc.vector.tensor_mul(out_sb[:], st["o_sb"][:], rep_ps[:])
                nc.sync.dma_start(
                    out_d.ap()[:, QB * qb : QB * (qb + 1)], out_sb[:]
                )

            def tail_c1(st):
                rep_ps = mpool.tile([97, 512], F32, name="rep97_ps", tag="p")
                nc.tensor.matmul(
                    rep_ps[:], ones97_sb[:], st["csr"][:], start=True, stop=True
                )
                st.update(rep_ps=rep_ps)

            def tail_c2(st):
                o2n_sb = workp.tile([97, 512], F16, name="o2n_sb")
                nc.vector.tensor_mul(o2n_sb[:], st["o2_sb"][:], st["rep_ps"][:])
                st.update(o2n_sb=o2n_sb)

            def tail_c3(st):
                qb = st["qb"]
                out_ps = mpool.tile([DH, 512], F32, name="out_ps", tag="p")
                nc.tensor.matmul(
                    out_ps[:], sel_sb[:], st["o2n_sb"][:], start=True, stop=True
                )
                out_sb = workp.tile([DH, 512], F32, name="out_sb")
                nc.vector.tensor_copy(out_sb[:], out_ps[:])
                nc.sync.dma_start(
                    out_d.ap()[:, QB * qb : QB * (qb + 1)], out_sb[:]
                )

            def emit_qb(qb):
                nkt = 4 * (qb + 1)
                ngr = (nkt + GS - 1) // GS
                o_ps = mpool.tile([97, 512], F32, name="o_ps", tag="o", bufs=1)
                if qb == 0:
                    # rows 33..63 are never written by PV but are contracted
                    # (x0) by the dn/merge matmuls: clear power-on NaN bits
                    # once; the single-buffer bank retains zeros afterwards
                    nc.vector.memset(o_ps[DH : 64, :], 0.0)
                pends = []

                def flush_pv(pend):
                    tiles, p_sb = pend
                    for (u, j) in tiles:
                        o = max(0, 128 * j - QB * qb)
                        b = (j % 2) if qb > 0 else 0
                        first = (j == 0) or (qb > 0 and j == 1)
                        last = (j == nkt - 2) if (qb > 0 and b == 0) else (
                            j == nkt - 1
                        )
                        nc.tensor.matmul(
                            o_ps[64 * b : 64 * b + DH + 1, o:512],
                            v_all[:, j, 0 : DH + 1],
                            p_sb[:, 512 * u + o : 512 * (u + 1)],
                            start=first,
                            stop=last,
                            skip_group_check=True,
                        )

                for g in range(ngr):
                    sz = min(GS, nkt - GS * g)
                    tiles = [(u, GS * g + u) for u in range(sz)]
                    s_ps = spool.tile([128, GS * 512], F32, name="s_ps", tag="s")
                    for (u, j) in tiles:
                        o = max(0, 128 * j - QB * qb)
                        nc.tensor.matmul(
                            s_ps[:, 512 * u + o : 512 * (u + 1)],
                            kT3[32 * u : 32 * u + 32, 128 * j : 128 * (j + 1)],
                            qT3[32 * u : 32 * u + 32, QB * qb + o : QB * (qb + 1)],
                            start=True, stop=True,
                        )
                    p_sb = workp.tile([128, GS * 512], F16, name="p_sb", bufs=8)
                    gidx = GIDX[0]
                    GIDX[0] += 1
                    if gidx >= 6 and (gidx - 6) % 4 == 0:
                        # exp on DVE via fp16 Schraudolph bit-hack:
                        # exp(s*SCALE) ~= bitcast_fp16(int16(s*(A*SCALE) + B));
                        # ~1.8% rms per-element error that averages out in the
                        # softmax ratio (calibrated p-weighted-mean-zero B)
                        with nc.allow_low_precision(reason="dve exp bit-hack"):
                            nc.vector.tensor_scalar(
                                p_sb[:, 0 : 512 * sz].bitcast(I16),
                                s_ps[:, 0 : 512 * sz],
                                HACK_A,
                                HACK_B,
                                op0=mybir.AluOpType.mult,
                                op1=mybir.AluOpType.add,
                            )
                    else:
                        nc.scalar.activation(
                            p_sb[:, 0 : 512 * sz],
                            s_ps[:, 0 : 512 * sz],
                            mybir.ActivationFunctionType.Exp,
                            scale=SCALE,
                        )
                    for (u, j) in tiles:
                        o = 128 * j - QB * qb
                        if o >= 0:  # strict-causal mask on the diagonal window
                            nc.gpsimd.tensor_mul(
                                p_sb[:, 512 * u + o : 512 * u + o + 128],
                                p_sb[:, 512 * u + o : 512 * u + o + 128],
                                tm_sb[:],
                            )
                    pends.append((tiles, p_sb))
                    if len(pends) > 2:
                        flush_pv(pends.pop(0))
                    if stage_q:
                        stage_q.pop(0)()
                while pends:
                    flush_pv(pends.pop(0))

                # tails: qb0 single-group path; qb>=1 merges the two
                # PV column groups with a select-matmul after normalizing in
                # fp16 (DVE cannot add across partitions)
                if qb == 0:
                    o33_sb = workp.tile([DH + 1, 512], F32, name="o33_sb")
                    nc.vector.tensor_scalar_add(o33_sb[:], o_ps[0 : DH + 1, :], eps_sb[:])
                    cs8 = workp.tile([8, 64], F32, name="cs8")
                    nc.sync.dma_start(cs8[:], o33_sb[DH : DH + 1, :])
                    st = {"qb": qb, "o_sb": o33_sb[0:DH, :], "cs8": cs8}
                    stage_q.append(lambda st=st: tail_b(st))
                    stage_q.append(lambda: None)
                    stage_q.append(lambda st=st: tail_c0(st))
                else:
                    o2_sb = workp.tile([97, 512], F16, name="o2_sb")
                    nc.vector.tensor_copy(o2_sb[:], o_ps[:])
                    dn_ps = mpool.tile([1, 512], F32, name="dn_ps", tag="p")
                    nc.tensor.matmul(
                        dn_ps[:], dsel_sb[:], o2_sb[:], start=True, stop=True
                    )
                    cs_sb = workp.tile([1, 512], F32, name="cs_sb")
                    nc.vector.tensor_copy(cs_sb[:], dn_ps[:])
                    cs8 = workp.tile([8, 64], F32, name="cs8")
                    nc.sync.dma_start(cs8[:], cs_sb[:])
                    st = {"qb": qb, "o2_sb": o2_sb, "cs8": cs8}
                    stage_q.append(lambda st=st: tail_b(st))
                    stage_q.append(lambda st=st: tail_c1(st))
                    stage_q.append(lambda st=st: tail_c2(st))
                    stage_q.append(lambda st=st: tail_c3(st))

            GIDX = [0]
            emit_kq(0)
            emit_kq(1)
            emit_v4(0)
            for qb in range(NQB):
                emit_qb(qb)
                if qb + 2 < NQB:
                    emit_kq(qb + 2)
                if qb + 1 < NQB:
                    emit_v4(qb + 1)
            while stage_q:
                stage_q.pop(0)()

    nc.finalize()
    return nc


def _get_nc():
    global _CACHED_NC
    if _CACHED_NC is None:
        _CACHED_NC = _build()
    return _CACHED_NC


def _prep_in_maps(inputs):
    f = lambda a: np.ascontiguousarray(np.asarray(a, dtype=np.float32))
    query = f(inputs["query"]).reshape(CQ, N)
    key_feat = f(inputs["key_feat"]).reshape(CK, N)

    def wnorm(v, g):
        v = f(v)
        g = f(g)
        return g[:, None] * v / np.linalg.norm(v, axis=1, keepdims=True)

    wq = wnorm(inputs["vq"], inputs["gq"])
    wk = wnorm(inputs["vk"], inputs["gk"])
    wv = wnorm(inputs["vv"], inputs["gv"])
    bq, bk, bv = f(inputs["bq"]), f(inputs["bk"]), f(inputs["bv"])

    in_maps = []
    for c in range(NH):
        rows = slice(DH * c, DH * (c + 1))
        in_maps.append(
            {
                "qin": query,
                "kin": key_feat,
                "wqt3": np.ascontiguousarray(np.tile(wq[rows].T, (1, 3))),
                "wkt3": np.ascontiguousarray(np.tile(wk[rows].T, (1, 3))),
                "wvt": np.ascontiguousarray(wv[rows].T),
                "bq3": np.ascontiguousarray(np.tile(bq[rows], 3)[:, None]),
                "bk3": np.ascontiguousarray(np.tile(bk[rows], 3)[:, None]),
                "bvr": np.ascontiguousarray(np.tile(bv[rows][None, :], (128, 4))),
            }
        )
    return in_maps


def _run(inputs, trace=False, **kwargs):
    nc = _get_nc()
    in_maps = _prep_in_maps(inputs)
    res = None
    for attempt in range(3):
        try:
            res = run_bass_kernel_spmd(
                nc, in_maps, core_ids=list(range(NH)), trace=trace, **kwargs
            )
            break
        except Exception:
            if attempt == 2:
                raise

    out = np.empty((B, CH, H, W), dtype=np.float32)
    for c in range(NH):
        oc = res.results[c]["out"]  # [DH, N] (O^T layout)
        out[0, DH * c : DH * (c + 1)] = oc.reshape(DH, H, W)
    return out, res


def kernel(**inputs) -> np.ndarray:
    out, _ = _run(inputs, trace=False)
    return out


# revision 29
# speedup vs baseline: 1.1476x; 1.1388x over previous
"""Trainium2 Bass kernel for nn_CausalAttention (N=4096, 8 heads, DH=32).

Strategy: head-parallel across 8 NeuronCores (1 head per core), tuned to be
ACT-engine bound (exp is only available on the scalar/ACT engine and its
~9.4M elements/core at 1 elem/lane/cycle @1.2GHz set a ~62us floor).

Per core:
  - PE warmup spin during the initial DMA window so the HAM clock gate
    reaches K=8/8 (2.4 GHz) before real matmuls start.
  - QK projections from channels-major inputs [256, 4096]; outputs written
    as 3x-replicated fp16 [96, N] (kT3/qT3) via bulk PSUM->SBUF
    tensor_scalar adds that fold the bias (one DVE op per 512-slice).
  - V projection in natural [key, d] orientation (kin slices as lhsT).
  - Scores S^T[k, q] = K Q^T in fp16, 3-way row-packed (tile_position via
    base partitions 0/32/64), 3 k-tiles per PSUM group (3 banks), strict
    causal: diagonal sub-tiles skip their fully-masked column prefix.
  - Max-free softmax: one exp activation per group [128, 512*nsub] on the
    ACT engine; nothing else runs on the scalar queue. Strict-causal 0/1
    mask applied post-exp on GpSimd. Garbage columns (masked prefixes) are
    never read: PV streams rhs starting at the prefix offset.
  - PV 2-way column-packed (col groups at partitions 0 and 64, 33-wide
    V|ones weights); denominator via the ones column; groups merged in the
    tail with one DVE add.
  - Normalization without transposes: per-block column sums -> reshape DMA
    [8,64] -> reciprocal -> [1,512] -> K=1 replicate matmul -> tensor_mul.
    Output stays in O^T layout [32, 4096] per core.
"""

import math

import numpy as np

import concourse.bass as bass
import concourse.mybir as mybir
from concourse import bacc
from concourse.tile import TileContext
from concourse.bass_utils import run_bass_kernel_spmd

# Problem constants (hardcoded per harness contract).
B, CQ, CK, CH, NH, H, W = 1, 256, 256, 256, 8, 64, 64
DH = CH // NH            # 32
N = H * W                # 4096
QB = 512                 # queries per block
NQB = N // QB            # 8
KT = 128                 # keys per k-tile
NKT = N // KT            # 32
GS = 3                   # k-tiles per score group (3 PSUM banks, 3-way row pack)
SCALE = 1.0 / math.sqrt(DH)
NWARM = 10               # PE warmup matmuls (HAM clock-gate warmup)
HACK_A = 1477.3195 / math.sqrt(DH)  # fp16 Schraudolph slope (folds 1/sqrt(DH))
HACK_B = 15301.0                    # p-weighted-mean-zero intercept

F32 = mybir.dt.float32
F32R = mybir.dt.float32r
F16 = mybir.dt.float16
I16 = mybir.dt.int16

_CACHED_NC = None


def _build():
    nc = bacc.Bacc("TRN2", target_bir_lowering=False, debug=False, num_devices=1)

    qin_d = nc.dram_tensor("qin", [CQ, N], F32, kind="ExternalInput")
    kin_d = nc.dram_tensor("kin", [CK, N], F32, kind="ExternalInput")
    wq_d = nc.dram_tensor("wqt3", [CQ, 96], F32, kind="ExternalInput")
    wk_d = nc.dram_tensor("wkt3", [CK, 96], F32, kind="ExternalInput")
    wv_d = nc.dram_tensor("wvt", [CK, DH], F32, kind="ExternalInput")
    bq_d = nc.dram_tensor("bq3", [96, 1], F32, kind="ExternalInput")
    bk_d = nc.dram_tensor("bk3", [96, 1], F32, kind="ExternalInput")
    bv_d = nc.dram_tensor("bvr", [128, 4 * DH], F32, kind="ExternalInput")
    out_d = nc.dram_tensor("out", [DH, N], F32, kind="ExternalOutput")

    # Strict-causal within-tile mask: tm[kk, qq] = 1.0 iff kk < qq.
    tm_np = (np.arange(128)[:, None] < np.arange(128)[None, :]).astype(np.float16)
    tm_d = nc.inline_tensor(tm_np, name="tmask")
    ones_d = nc.inline_tensor(np.ones((1, DH), dtype=np.float32), name="onesd")
    eps_np = np.zeros((DH + 1, 1), dtype=np.float32)
    eps_np[DH, 0] = 1e-30
    eps_d = nc.inline_tensor(eps_np, name="epsd")
    sel_np = np.zeros((97, DH), dtype=np.float16)
    for _d in range(DH):
        sel_np[_d, _d] = 1.0
        sel_np[64 + _d, _d] = 1.0
    sel_d = nc.inline_tensor(sel_np, name="seld")
    dsel_np = np.zeros((97, 1), dtype=np.float16)
    dsel_np[DH, 0] = 1.0
    dsel_np[96, 0] = 1.0
    dsel_d = nc.inline_tensor(dsel_np, name="dseld")
    ones97_d = nc.inline_tensor(np.ones((1, 97), dtype=np.float32), name="ones97d")

    kin_ap = kin_d.ap().rearrange("(c p) n -> p c n", p=128).bitcast(F32R)
    qin_ap = qin_d.ap().rearrange("(c p) n -> p c n", p=128).bitcast(F32R)
    wk_ap = wk_d.ap().rearrange("(c p) m -> p c m", p=128).bitcast(F32R)
    wq_ap = wq_d.ap().rearrange("(c p) m -> p c m", p=128).bitcast(F32R)
    wv_ap = wv_d.ap().rearrange("(c p) m -> p c m", p=128).bitcast(F32R)

    with TileContext(nc) as tc:
        with (
            tc.tile_pool(name="constp", bufs=1) as constp,
            tc.tile_pool(name="bigp", bufs=1) as bigp,
            tc.tile_pool(name="workp", bufs=4) as workp,
            tc.tile_pool(name="spool", bufs=2, space="PSUM") as spool,
            tc.tile_pool(name="mpool", bufs=1, space="PSUM") as mpool,
        ):
            # ---- big SBUF tiles ----
            kin_sb = bigp.tile([128, 2, N], F32R, name="kin_sb")
            qin_sb = bigp.tile([128, 2, N], F32R, name="qin_sb")
            kT3 = bigp.tile([96, N], F16, name="kT3")    # 3x replicated k^T
            qT3 = bigp.tile([96, N], F16, name="qT3")    # 3x replicated q^T
            # v_all[kk, t, 0:DH] = v[128t+kk, :]; col DH is the ones column
            v_all = bigp.tile([128, NKT, 48], F16, name="v_all")
            warm = bigp.tile([32, 640], F16, name="warm")

            # ---- DMAs: weights + first slices first; kin on sync, qin on
            # gpsimd (cheap issue), nothing on the scalar queue ----
            wk_sb = constp.tile([128, 2, 96], F32R, name="wk_sb")
            nc.sync.dma_start(wk_sb[:], wk_ap)
            wq_sb = constp.tile([128, 2, 96], F32R, name="wq_sb")
            nc.sync.dma_start(wq_sb[:], wq_ap)
            slA = slice(0, QB)
            nc.sync.dma_start(kin_sb[:, :, slA], kin_ap[:, :, slA])
            nc.sync.dma_start(qin_sb[:, :, slA], qin_ap[:, :, slA])
            wv_sb = constp.tile([128, 2, DH], F32R, name="wv_sb")
            nc.gpsimd.dma_start(wv_sb[:], wv_ap)
            bq_sb = constp.tile([96, 1], F32, name="bq_sb")
            nc.gpsimd.dma_start(bq_sb[:], bq_d.ap())
            bk_sb = constp.tile([96, 1], F32, name="bk_sb")
            nc.gpsimd.dma_start(bk_sb[:], bk_d.ap())
            bv_sb = constp.tile([128, 4, DH], F32, name="bv_sb")
            nc.gpsimd.dma_start(bv_sb[:], bv_d.ap().rearrange("p (t d) -> p t d", t=4))
            tm_sb = constp.tile([128, 128], F16, name="tm_sb")
            nc.gpsimd.dma_start(tm_sb[:], tm_d.ap())
            ones_sb = constp.tile([1, DH], F32R, name="ones_sb")
            nc.gpsimd.dma_start(ones_sb[:], ones_d.ap().bitcast(F32R))
            eps_sb = constp.tile([DH + 1, 1], F32, name="eps_sb")
            nc.gpsimd.dma_start(eps_sb[:], eps_d.ap())
            sel_sb = constp.tile([97, DH], F16, name="sel_sb")
            nc.gpsimd.dma_start(sel_sb[:], sel_d.ap())
            dsel_sb = constp.tile([97, 1], F16, name="dsel_sb")
            nc.gpsimd.dma_start(dsel_sb[:], dsel_d.ap())
            ones97_sb = constp.tile([1, 97], F32R, name="ones97_sb")
            nc.gpsimd.dma_start(ones97_sb[:], ones97_d.ap().bitcast(F32R))
            nc.vector.memset(warm[:], 0.0)
            nc.vector.memset(v_all[:, :, DH : DH + 1], 1.0)
            for s in range(1, NQB):
                sl = slice(QB * s, QB * (s + 1))
                nc.gpsimd.dma_start(kin_sb[:, :, sl], kin_ap[:, :, sl])
                nc.gpsimd.dma_start(qin_sb[:, :, sl], qin_ap[:, :, sl])

            # ---- PE warmup: keep the array busy through the DMA window so
            # the HAM un-throttles before real matmuls arrive ----
            wm_ps = mpool.tile([128, 512], F32, name="wm_ps", tag="p")
            for i in range(NWARM):
                nc.tensor.matmul(
                    wm_ps[:], warm[0:32, 0:128], warm[0:32, 128:640],
                    start=(i == 0), stop=(i == NWARM - 1),
                )

            stage_q = []  # deferred tail stages, advanced one per score group

            def emit_kq(s):
                ksl = slice(QB * s, QB * (s + 1))
                pjk = mpool.tile([96, 512], F32, name="pjk", tag="p")
                for ch in range(2):
                    nc.tensor.matmul(
                        pjk[:], wk_sb[:, ch, :], kin_sb[:, ch, ksl],
                        start=(ch == 0), stop=(ch == 1),
                    )
                nc.vector.tensor_scalar_add(kT3[:, ksl], pjk[:], bk_sb[:])
                pjq = mpool.tile([96, 512], F32, name="pjq", tag="p")
                for ch in range(2):
                    nc.tensor.matmul(
                        pjq[:], wq_sb[:, ch, :], qin_sb[:, ch, ksl],
                        start=(ch == 0), stop=(ch == 1),
                    )
                nc.vector.tensor_scalar_add(qT3[:, ksl], pjq[:], bq_sb[:])

            def emit_v4(s):
                # 4 v-tiles of one slice into one PSUM tile: no per-tile DVE
                # round-trips on the single-bank "p" ring, one batched add
                pv4 = mpool.tile([128, 4, DH], F32, name="pv4", tag="p")
                for ti in range(4):
                    t = 4 * s + ti
                    nsl = slice(128 * t, 128 * (t + 1))
                    for ch in range(2):
                        nc.tensor.matmul(
                            pv4[:, ti, :], kin_sb[:, ch, nsl], wv_sb[:, ch, :],
                            start=(ch == 0), stop=(ch == 1),
                        )
                nc.vector.tensor_add(
                    v_all[:, 4 * s : 4 * s + 4, 0:DH], pv4[:], bv_sb[:]
                )

            def tail_b(st):
                cs8r = workp.tile([8, 64], F32, name="cs8r")
                nc.vector.reciprocal(cs8r[:], st["cs8"][:])
                csr = workp.tile([1, 512], F32R, name="csr")
                nc.sync.dma_start(csr[:], cs8r[:].bitcast(F32R))
                st.update(csr=csr)

            def tail_c0(st):
                qb = st["qb"]
                rep_ps = mpool.tile([DH, 512], F32, name="rep_ps", tag="p")
                nc.tensor.matmul(
                    rep_ps[:], ones_sb[:], st["csr"][:], start=True, stop=True
                )
                out_sb = workp.tile([DH, 512], F32, name="out_sb")
                nc.vector.tensor_mul(out_sb[:], st["o_sb"][:], rep_ps[:])
                nc.sync.dma_start(
                    out_d.ap()[:, QB * qb : QB * (qb + 1)], out_sb[:]
                )

            def tail_c1(st):
                rep_ps = mpool.tile([97, 512], F32, name="rep97_ps", tag="p")
                nc.tensor.matmul(
                    rep_ps[:], ones97_sb[:], st["csr"][:], start=True, stop=True
                )
                st.update(rep_ps=rep_ps)

            def tail_c2(st):
                o2n_sb = workp.tile([97, 512], F16, name="o2n_sb")
                nc.vector.tensor_mul(o2n_sb[:], st["o2_sb"][:], st["rep_ps"][:])
                st.update(o2n_sb=o2n_sb)

            def tail_c3(st):
                qb = st["qb"]
                out_ps = mpool.tile([DH, 512], F32, name="out_ps", tag="p")
                nc.tensor.matmul(
                    out_ps[:], sel_sb[:], st["o2n_sb"][:], start=True, stop=True
                )
                out_sb = workp.tile([DH, 512], F32, name="out_sb")
                nc.vector.tensor_copy(out_sb[:], out_ps[:])
                nc.sync.dma_start(
                    out_d.ap()[:, QB * qb : QB * (qb + 1)], out_sb[:]
                )

            def emit_qb(qb):
                nkt = 4 * (qb + 1)
                ngr = (nkt + GS - 1) // GS
                o_ps = mpool.tile([DH + 1, 512], F32, name="o_ps", tag="o", bufs=1)
                pends = []

                def flush_pv(pend):
                    tiles, p_sb = pend
                    for (u, j) in tiles:
                        o = max(0, 128 * j - QB * qb)
                        nc.tensor.matmul(
                            o_ps[:, o:512],
                            v_all[:, j, 0 : DH + 1],
                            p_sb[:, 512 * u + o : 512 * (u + 1)],
                            start=(j == 0),
                            stop=(j == nkt - 1),
                            skip_group_check=True,
                        )

                for g in range(ngr):
                    sz = min(GS, nkt - GS * g)
                    tiles = [(u, GS * g + u) for u in range(sz)]
                    s_ps = spool.tile([128, GS * 512], F32, name="s_ps", tag="s")
                    for (u, j) in tiles:
                        o = max(0, 128 * j - QB * qb)
                        nc.tensor.matmul(
                            s_ps[:, 512 * u + o : 512 * (u + 1)],
                            kT3[32 * u : 32 * u + 32, 128 * j : 128 * (j + 1)],
                            qT3[32 * u : 32 * u + 32, QB * qb + o : QB * (qb + 1)],
                            start=True, stop=True,
                        )
                    p_sb = workp.tile([128, GS * 512], F16, name="p_sb", bufs=8)
                    gidx = GIDX[0]
                    GIDX[0] += 1
                    if gidx >= 6 and (gidx - 6) % 4 == 0:
                        # exp on DVE via fp16 Schraudolph bit-hack:
                        # exp(s*SCALE) ~= bitcast_fp16(int16(s*(A*SCALE) + B));
                        # ~1.8% rms per-element error that averages out in the
                        # softmax ratio (calibrated p-weighted-mean-zero B)
                        with nc.allow_low_precision(reason="dve exp bit-hack"):
                            nc.vector.tensor_scalar(
                                p_sb[:, 0 : 512 * sz].bitcast(I16),
                                s_ps[:, 0 : 512 * sz],
                                HACK_A,
                                HACK_B,
                                op0=mybir.AluOpType.mult,
                                op1=mybir.AluOpType.add,
                            )
                    else:
                        nc.scalar.activation(
                            p_sb[:, 0 : 512 * sz],
                            s_ps[:, 0 : 512 * sz],
                            mybir.ActivationFunctionType.Exp,
                            scale=SCALE,
                        )
                    for (u, j) in tiles:
                        o = 128 * j - QB * qb
                        if o >= 0:  # strict-causal mask on the diagonal window
                            nc.gpsimd.tensor_mul(
                                p_sb[:, 512 * u + o : 512 * u + o + 128],
                                p_sb[:, 512 * u + o : 512 * u + o + 128],
                                tm_sb[:],
                            )
                    pends.append((tiles, p_sb))
                    if len(pends) > 2:
                        flush_pv(pends.pop(0))
                    if stage_q:
                        stage_q.pop(0)()
                while pends:
                    flush_pv(pends.pop(0))

                # tail_a inline: one fused PSUM->SBUF copy; the eps
                # column adds 1e-30 only to the denominator row (keeps q=0 at
                # 0 instead of NaN)
                o33_sb = workp.tile([DH + 1, 512], F32, name="o33_sb")
                nc.vector.tensor_scalar_add(o33_sb[:], o_ps[:], eps_sb[:])
                cs8 = workp.tile([8, 64], F32, name="cs8")
                nc.sync.dma_start(cs8[:], o33_sb[DH : DH + 1, :])
                st = {"qb": qb, "o_sb": o33_sb[0:DH, :], "cs8": cs8}
                stage_q.append(lambda st=st: tail_b(st))
                stage_q.append(lambda: None)
                stage_q.append(lambda st=st: tail_c0(st))

            GIDX = [0]
            emit_kq(0)
            emit_kq(1)
            emit_v4(0)
            for qb in range(NQB):
                emit_qb(qb)
                if qb + 2 < NQB:
                    emit_kq(qb + 2)
                if qb + 1 < NQB:
                    emit_v4(qb + 1)
            while stage_q:
                stage_q.pop(0)()

    nc.finalize()
    return nc


def _get_nc():
    global _CACHED_NC
    if _CACHED_NC is None:
        _CACHED_NC = _build()
    return _CACHED_NC


def _prep_in_maps(inputs):
    f = lambda a: np.ascontiguousarray(np.asarray(a, dtype=np.float32))
    query = f(inputs["query"]).reshape(CQ, N)
    key_feat = f(inputs["key_feat"]).reshape(CK, N)

    def wnorm(v, g):
        v = f(v)
        g = f(g)
        return g[:, None] * v / np.linalg.norm(v, axis=1, keepdims=True)

    wq = wnorm(inputs["vq"], inputs["gq"])
    wk = wnorm(inputs["vk"], inputs["gk"])
    wv = wnorm(inputs["vv"], inputs["gv"])
    bq, bk, bv = f(inputs["bq"]), f(inputs["bk"]), f(inputs["bv"])

    in_maps = []
    for c in range(NH):
        rows = slice(DH * c, DH * (c + 1))
        in_maps.append(
            {
                "qin": query,
                "kin": key_feat,
                "wqt3": np.ascontiguousarray(np.tile(wq[rows].T, (1, 3))),
                "wkt3": np.ascontiguousarray(np.tile(wk[rows].T, (1, 3))),
                "wvt": np.ascontiguousarray(wv[rows].T),
                "bq3": np.ascontiguousarray(np.tile(bq[rows], 3)[:, None]),
                "bk3": np.ascontiguousarray(np.tile(bk[rows], 3)[:, None]),
                "bvr": np.ascontiguousarray(np.tile(bv[rows][None, :], (128, 4))),
            }
        )
    return in_maps


def _run(inputs, trace=False, **kwargs):
    nc = _get_nc()
    in_maps = _prep_in_maps(inputs)
    res = None
    for attempt in range(3):
        try:
            res = run_bass_kernel_spmd(
                nc, in_maps, core_ids=list(range(NH)), trace=trace, **kwargs
            )
            break
        except Exception:
            if attempt == 2:
                raise

    out = np.empty((B, CH, H, W), dtype=np.float32)
    for c in range(NH):
        oc = res.results[c]["out"]  # [DH, N] (O^T layout)
        out[0, DH * c : DH * (c + 1)] = oc.reshape(DH, H, W)
    return out, res


def kernel(**inputs) -> np.ndarray:
    out, _ = _run(inputs, trace=False)
    return out


# revision 30
# speedup vs baseline: 1.2012x; 1.0467x over previous
"""Trainium2 Bass kernel for nn_CausalAttention (N=4096, 8 heads, DH=32).

Strategy: head-parallel across 8 NeuronCores (1 head per core), tuned to be
ACT-engine bound (exp is only available on the scalar/ACT engine and its
~9.4M elements/core at 1 elem/lane/cycle @1.2GHz set a ~62us floor).

Per core:
  - PE warmup spin during the initial DMA window so the HAM clock gate
    reaches K=8/8 (2.4 GHz) before real matmuls start.
  - QK projections from channels-major inputs [256, 4096]; outputs written
    as 3x-replicated fp16 [96, N] (kT3/qT3) via bulk PSUM->SBUF
    tensor_scalar adds that fold the bias (one DVE op per 512-slice).
  - V projection in natural [key, d] orientation (kin slices as lhsT).
  - Scores S^T[k, q] = K Q^T in fp16, 3-way row-packed (tile_position via
    base partitions 0/32/64), 3 k-tiles per PSUM group (3 banks), strict
    causal: diagonal sub-tiles skip their fully-masked column prefix.
  - Max-free softmax: one exp activation per group [128, 512*nsub] on the
    ACT engine; nothing else runs on the scalar queue. Strict-causal 0/1
    mask applied post-exp on GpSimd. Garbage columns (masked prefixes) are
    never read: PV streams rhs starting at the prefix offset.
  - PV 2-way column-packed (col groups at partitions 0 and 64, 33-wide
    V|ones weights); denominator via the ones column; groups merged in the
    tail with one DVE add.
  - Normalization without transposes: per-block column sums -> reshape DMA
    [8,64] -> reciprocal -> [1,512] -> K=1 replicate matmul -> tensor_mul.
    Output stays in O^T layout [32, 4096] per core.
"""

import math

import numpy as np

import concourse.bass as bass
import concourse.mybir as mybir
from concourse import bacc
from concourse.tile import TileContext
from concourse.bass_utils import run_bass_kernel_spmd

# Problem constants (hardcoded per harness contract).
B, CQ, CK, CH, NH, H, W = 1, 256, 256, 256, 8, 64, 64
DH = CH // NH            # 32
N = H * W                # 4096
QB = 512                 # queries per block
NQB = N // QB            # 8
KT = 128                 # keys per k-tile
NKT = N // KT            # 32
GS = 3                   # k-tiles per score group (3 PSUM banks, 3-way row pack)
SCALE = 1.0 / math.sqrt(DH)
NWARM = 10               # PE warmup matmuls (HAM clock-gate warmup)
HACK_A = 1477.3195 / math.sqrt(DH)  # fp16 Schraudolph slope (folds 1/sqrt(DH))
HACK_B = 15301.0                    # p-weighted-mean-zero intercept

F32 = mybir.dt.float32
F32R = mybir.dt.float32r
F16 = mybir.dt.float16
I16 = mybir.dt.int16

_CACHED_NC = None


def _build():
    nc = bacc.Bacc("TRN2", target_bir_lowering=False, debug=False, num_devices=1)

    qin_d = nc.dram_tensor("qin", [CQ, N], F32, kind="ExternalInput")
    kin_d = nc.dram_tensor("kin", [CK, N], F32, kind="ExternalInput")
    wq_d = nc.dram_tensor("wqt3", [CQ, 96], F32, kind="ExternalInput")
    wk_d = nc.dram_tensor("wkt3", [CK, 96], F32, kind="ExternalInput")
    wv_d = nc.dram_tensor("wvt", [CK, DH], F32, kind="ExternalInput")
    bq_d = nc.dram_tensor("bq3", [96, 1], F32, kind="ExternalInput")
    bk_d = nc.dram_tensor("bk3", [96, 1], F32, kind="ExternalInput")
    bv_d = nc.dram_tensor("bvr", [128, 4 * DH], F32, kind="ExternalInput")
    out_d = nc.dram_tensor("out", [DH, N], F32, kind="ExternalOutput")

    # Strict-causal within-tile mask: tm[kk, qq] = 1.0 iff kk < qq.
    tm_np = (np.arange(128)[:, None] < np.arange(128)[None, :]).astype(np.float16)
    tm_d = nc.inline_tensor(tm_np, name="tmask")
    ones_d = nc.inline_tensor(np.ones((1, DH), dtype=np.float32), name="onesd")
    eps_np = np.zeros((DH + 1, 1), dtype=np.float32)
    eps_np[DH, 0] = 1e-30
    eps_d = nc.inline_tensor(eps_np, name="epsd")
    sel_np = np.zeros((97, DH), dtype=np.float16)
    for _d in range(DH):
        sel_np[_d, _d] = 1.0
        sel_np[64 + _d, _d] = 1.0
    sel_d = nc.inline_tensor(sel_np, name="seld")
    dsel_np = np.zeros((97, 1), dtype=np.float16)
    dsel_np[DH, 0] = 1.0
    dsel_np[96, 0] = 1.0
    dsel_d = nc.inline_tensor(dsel_np, name="dseld")
    ones97_d = nc.inline_tensor(np.ones((1, 97), dtype=np.float32), name="ones97d")

    kin_ap = kin_d.ap().rearrange("(c p) n -> p c n", p=128).bitcast(F32R)
    qin_ap = qin_d.ap().rearrange("(c p) n -> p c n", p=128).bitcast(F32R)
    wk_ap = wk_d.ap().rearrange("(c p) m -> p c m", p=128).bitcast(F32R)
    wq_ap = wq_d.ap().rearrange("(c p) m -> p c m", p=128).bitcast(F32R)
    wv_ap = wv_d.ap().rearrange("(c p) m -> p c m", p=128).bitcast(F32R)

    with TileContext(nc) as tc:
        with (
            tc.tile_pool(name="constp", bufs=1) as constp,
            tc.tile_pool(name="bigp", bufs=1) as bigp,
            tc.tile_pool(name="workp", bufs=4) as workp,
            tc.tile_pool(name="spool", bufs=2, space="PSUM") as spool,
            tc.tile_pool(name="mpool", bufs=1, space="PSUM") as mpool,
        ):
            # ---- big SBUF tiles ----
            kin_sb = bigp.tile([128, 2, N], F32R, name="kin_sb")
            qin_sb = bigp.tile([128, 2, N], F32R, name="qin_sb")
            kT3 = bigp.tile([96, N], F16, name="kT3")    # 3x replicated k^T
            qT3 = bigp.tile([96, N], F16, name="qT3")    # 3x replicated q^T
            # v_all[kk, t, 0:DH] = v[128t+kk, :]; col DH is the ones column
            v_all = bigp.tile([128, NKT, 48], F16, name="v_all")
            warm = bigp.tile([32, 640], F16, name="warm")

            # ---- DMAs: weights + first slices first; kin on sync, qin on
            # gpsimd (cheap issue), nothing on the scalar queue ----
            wk_sb = constp.tile([128, 2, 96], F32R, name="wk_sb")
            nc.sync.dma_start(wk_sb[:], wk_ap)
            wq_sb = constp.tile([128, 2, 96], F32R, name="wq_sb")
            nc.sync.dma_start(wq_sb[:], wq_ap)
            slA = slice(0, QB)
            nc.sync.dma_start(kin_sb[:, :, slA], kin_ap[:, :, slA])
            nc.sync.dma_start(qin_sb[:, :, slA], qin_ap[:, :, slA])
            wv_sb = constp.tile([128, 2, DH], F32R, name="wv_sb")
            nc.gpsimd.dma_start(wv_sb[:], wv_ap)
            bq_sb = constp.tile([96, 1], F32, name="bq_sb")
            nc.gpsimd.dma_start(bq_sb[:], bq_d.ap())
            bk_sb = constp.tile([96, 1], F32, name="bk_sb")
            nc.gpsimd.dma_start(bk_sb[:], bk_d.ap())
            bv_sb = constp.tile([128, 4, DH], F32, name="bv_sb")
            nc.gpsimd.dma_start(bv_sb[:], bv_d.ap().rearrange("p (t d) -> p t d", t=4))
            tm_sb = constp.tile([128, 128], F16, name="tm_sb")
            nc.gpsimd.dma_start(tm_sb[:], tm_d.ap())
            ones_sb = constp.tile([1, DH], F32R, name="ones_sb")
            nc.gpsimd.dma_start(ones_sb[:], ones_d.ap().bitcast(F32R))
            eps_sb = constp.tile([DH + 1, 1], F32, name="eps_sb")
            nc.gpsimd.dma_start(eps_sb[:], eps_d.ap())
            sel_sb = constp.tile([97, DH], F16, name="sel_sb")
            nc.gpsimd.dma_start(sel_sb[:], sel_d.ap())
            dsel_sb = constp.tile([97, 1], F16, name="dsel_sb")
            nc.gpsimd.dma_start(dsel_sb[:], dsel_d.ap())
            ones97_sb = constp.tile([1, 97], F32R, name="ones97_sb")
            nc.gpsimd.dma_start(ones97_sb[:], ones97_d.ap().bitcast(F32R))
            nc.vector.memset(warm[:], 0.0)
            nc.vector.memset(v_all[:, :, DH : DH + 1], 1.0)
            for s in range(1, NQB):
                sl = slice(QB * s, QB * (s + 1))
                nc.gpsimd.dma_start(kin_sb[:, :, sl], kin_ap[:, :, sl])
                nc.gpsimd.dma_start(qin_sb[:, :, sl], qin_ap[:, :, sl])

            # ---- PE warmup: keep the array busy through the DMA window so
            # the HAM un-throttles before real matmuls arrive ----
            wm_ps = mpool.tile([128, 512], F32, name="wm_ps", tag="p")
            for i in range(NWARM):
                nc.tensor.matmul(
                    wm_ps[:], warm[0:32, 0:128], warm[0:32, 128:640],
                    start=(i == 0), stop=(i == NWARM - 1),
                )

            stage_q = []  # deferred tail stages, advanced one per score group

            def emit_kq(s):
                ksl = slice(QB * s, QB * (s + 1))
                pjk = mpool.tile([96, 512], F32, name="pjk", tag="p")
                for ch in range(2):
                    nc.tensor.matmul(
                        pjk[:], wk_sb[:, ch, :], kin_sb[:, ch, ksl],
                        start=(ch == 0), stop=(ch == 1),
                    )
                nc.vector.tensor_scalar_add(kT3[:, ksl], pjk[:], bk_sb[:])
                pjq = mpool.tile([96, 512], F32, name="pjq", tag="p")
                for ch in range(2):
                    nc.tensor.matmul(
                        pjq[:], wq_sb[:, ch, :], qin_sb[:, ch, ksl],
                        start=(ch == 0), stop=(ch == 1),
                    )
                nc.vector.tensor_scalar_add(qT3[:, ksl], pjq[:], bq_sb[:])

            def emit_v4(s):
                # 4 v-tiles of one slice into one PSUM tile: no per-tile DVE
                # round-trips on the single-bank "p" ring, one batched add
                pv4 = mpool.tile([128, 4, DH], F32, name="pv4", tag="p")
                for ti in range(4):
                    t = 4 * s + ti
                    nsl = slice(128 * t, 128 * (t + 1))
                    for ch in range(2):
                        nc.tensor.matmul(
                            pv4[:, ti, :], kin_sb[:, ch, nsl], wv_sb[:, ch, :],
                            start=(ch == 0), stop=(ch == 1),
                        )
                nc.vector.tensor_add(
                    v_all[:, 4 * s : 4 * s + 4, 0:DH], pv4[:], bv_sb[:]
                )

            def tail_b(st):
                cs8r = workp.tile([8, 64], F32, name="cs8r")
                nc.vector.reciprocal(cs8r[:], st["cs8"][:])
                csr = workp.tile([1, 512], F32R, name="csr")
                nc.sync.dma_start(csr[:], cs8r[:].bitcast(F32R))
                st.update(csr=csr)

            def tail_c0(st):
                qb = st["qb"]
                rep_ps = mpool.tile([DH, 512], F32, name="rep_ps", tag="p")
                nc.tensor.matmul(
                    rep_ps[:], ones_sb[:], st["csr"][:], start=True, stop=True
                )
                out_sb = workp.tile([DH, 512], F32, name="out_sb")
                nc.vector.tensor_mul(out_sb[:], st["o_sb"][:], rep_ps[:])
                nc.sync.dma_start(
                    out_d.ap()[:, QB * qb : QB * (qb + 1)], out_sb[:]
                )

            def tail_c1(st):
                rep_ps = mpool.tile([97, 512], F32, name="rep97_ps", tag="p")
                nc.tensor.matmul(
                    rep_ps[:], ones97_sb[:], st["csr"][:], start=True, stop=True
                )
                st.update(rep_ps=rep_ps)

            def tail_c2(st):
                o2n_sb = workp.tile([97, 512], F16, name="o2n_sb")
                nc.vector.tensor_mul(o2n_sb[:], st["o2_sb"][:], st["rep_ps"][:])
                st.update(o2n_sb=o2n_sb)

            def tail_c3(st):
                qb = st["qb"]
                out_ps = mpool.tile([DH, 512], F32, name="out_ps", tag="p")
                nc.tensor.matmul(
                    out_ps[:], sel_sb[:], st["o2n_sb"][:], start=True, stop=True
                )
                out_sb = workp.tile([DH, 512], F32, name="out_sb")
                nc.vector.tensor_copy(out_sb[:], out_ps[:])
                nc.sync.dma_start(
                    out_d.ap()[:, QB * qb : QB * (qb + 1)], out_sb[:]
                )

            pends = []  # ('pv', tiles, p_sb, o_ps, qb, nkt) | ('tail', qb, o_ps)

            def pump():
                if not pends:
                    return
                e = pends.pop(0)
                if e[0] == "pv":
                    _, tiles, p_sb, o_ps, qb, nkt = e
                    for (u, j) in tiles:
                        o = max(0, 128 * j - QB * qb)
                        nc.tensor.matmul(
                            o_ps[:, o:512],
                            v_all[:, j, 0 : DH + 1],
                            p_sb[:, 512 * u + o : 512 * (u + 1)],
                            start=(j == 0),
                            stop=(j == nkt - 1),
                            skip_group_check=True,
                        )
                else:
                    _, qb, o_ps = e
                    o33_sb = workp.tile([DH + 1, 512], F32, name="o33_sb")
                    nc.vector.tensor_scalar_add(o33_sb[:], o_ps[:], eps_sb[:])
                    cs8 = workp.tile([8, 64], F32, name="cs8")
                    nc.sync.dma_start(cs8[:], o33_sb[DH : DH + 1, :])
                    st = {"qb": qb, "o_sb": o33_sb[0:DH, :], "cs8": cs8}
                    stage_q.append(lambda st=st: tail_b(st))
                    stage_q.append(lambda: None)
                    stage_q.append(lambda st=st: tail_c0(st))

            def emit_qb(qb):
                nkt = 4 * (qb + 1)
                ngr = (nkt + GS - 1) // GS
                o_ps = mpool.tile([DH + 1, 512], F32, name="o_ps", tag="o", bufs=1)

                for g in range(ngr):
                    sz = min(GS, nkt - GS * g)
                    tiles = [(u, GS * g + u) for u in range(sz)]
                    s_ps = spool.tile([128, GS * 512], F32, name="s_ps", tag="s")
                    for (u, j) in tiles:
                        o = max(0, 128 * j - QB * qb)
                        nc.tensor.matmul(
                            s_ps[:, 512 * u + o : 512 * (u + 1)],
                            kT3[32 * u : 32 * u + 32, 128 * j : 128 * (j + 1)],
                            qT3[32 * u : 32 * u + 32, QB * qb + o : QB * (qb + 1)],
                            start=True, stop=True,
                        )
                    p_sb = workp.tile([128, GS * 512], F16, name="p_sb", bufs=8)
                    gidx = GIDX[0]
                    GIDX[0] += 1
                    if gidx >= 6 and (gidx - 6) % 4 == 0:
                        # exp on DVE via fp16 Schraudolph bit-hack:
                        # exp(s*SCALE) ~= bitcast_fp16(int16(s*(A*SCALE) + B))
                        with nc.allow_low_precision(reason="dve exp bit-hack"):
                            nc.vector.tensor_scalar(
                                p_sb[:, 0 : 512 * sz].bitcast(I16),
                                s_ps[:, 0 : 512 * sz],
                                HACK_A,
                                HACK_B,
                                op0=mybir.AluOpType.mult,
                                op1=mybir.AluOpType.add,
                            )
                    else:
                        nc.scalar.activation(
                            p_sb[:, 0 : 512 * sz],
                            s_ps[:, 0 : 512 * sz],
                            mybir.ActivationFunctionType.Exp,
                            scale=SCALE,
                        )
                    for (u, j) in tiles:
                        o = 128 * j - QB * qb
                        if o >= 0:  # strict-causal mask on the diagonal window
                            nc.gpsimd.tensor_mul(
                                p_sb[:, 512 * u + o : 512 * u + o + 128],
                                p_sb[:, 512 * u + o : 512 * u + o + 128],
                                tm_sb[:],
                            )
                    pends.append(("pv", tiles, p_sb, o_ps, qb, nkt))
                    while len(pends) > 2:
                        pump()
                    if stage_q:
                        stage_q.pop(0)()
                pends.append(("tail", qb, o_ps))

            GIDX = [0]
            emit_kq(0)
            emit_kq(1)
            emit_v4(0)
            for qb in range(NQB):
                emit_qb(qb)
                if qb + 2 < NQB:
                    emit_kq(qb + 2)
                if qb + 1 < NQB:
                    emit_v4(qb + 1)
            while pends:
                pump()
            while stage_q:
                stage_q.pop(0)()

    nc.finalize()
    return nc


def _get_nc():
    global _CACHED_NC
    if _CACHED_NC is None:
        _CACHED_NC = _build()
    return _CACHED_NC


def _prep_in_maps(inputs):
    f = lambda a: np.ascontiguousarray(np.asarray(a, dtype=np.float32))
    query = f(inputs["query"]).reshape(CQ, N)
    key_feat = f(inputs["key_feat"]).reshape(CK, N)

    def wnorm(v, g):
        v = f(v)
        g = f(g)
        return g[:, None] * v / np.linalg.norm(v, axis=1, keepdims=True)

    wq = wnorm(inputs["vq"], inputs["gq"])
    wk = wnorm(inputs["vk"], inputs["gk"])
    wv = wnorm(inputs["vv"], inputs["gv"])
    bq, bk, bv = f(inputs["bq"]), f(inputs["bk"]), f(inputs["bv"])

    in_maps = []
    for c in range(NH):
        rows = slice(DH * c, DH * (c + 1))
        in_maps.append(
            {
                "qin": query,
                "kin": key_feat,
                "wqt3": np.ascontiguousarray(np.tile(wq[rows].T, (1, 3))),
                "wkt3": np.ascontiguousarray(np.tile(wk[rows].T, (1, 3))),
                "wvt": np.ascontiguousarray(wv[rows].T),
                "bq3": np.ascontiguousarray(np.tile(bq[rows], 3)[:, None]),
                "bk3": np.ascontiguousarray(np.tile(bk[rows], 3)[:, None]),
                "bvr": np.ascontiguousarray(np.tile(bv[rows][None, :], (128, 4))),
            }
        )
    return in_maps


def _run(inputs, trace=False, **kwargs):
    nc = _get_nc()
    in_maps = _prep_in_maps(inputs)
    res = None
    for attempt in range(3):
        try:
            res = run_bass_kernel_spmd(
                nc, in_maps, core_ids=list(range(NH)), trace=trace, **kwargs
            )
            break
        except Exception:
            if attempt == 2:
                raise

    out = np.empty((B, CH, H, W), dtype=np.float32)
    for c in range(NH):
        oc = res.results[c]["out"]  # [DH, N] (O^T layout)
        out[0, DH * c : DH * (c + 1)] = oc.reshape(DH, H, W)
    return out, res


def kernel(**inputs) -> np.ndarray:
    out, _ = _run(inputs, trace=False)
    return out


# revision 31
# speedup vs baseline: 1.2803x; 1.0659x over previous
"""Trainium2 Bass kernel for nn_CausalAttention (N=4096, 8 heads, DH=32).

Strategy: head-parallel across 8 NeuronCores (1 head per core), tuned to be
ACT-engine bound (exp is only available on the scalar/ACT engine and its
~9.4M elements/core at 1 elem/lane/cycle @1.2GHz set a ~62us floor).

Per core:
  - PE warmup spin during the initial DMA window so the HAM clock gate
    reaches K=8/8 (2.4 GHz) before real matmuls start.
  - QK projections from channels-major inputs [256, 4096]; outputs written
    as 3x-replicated fp16 [96, N] (kT3/qT3) via bulk PSUM->SBUF
    tensor_scalar adds that fold the bias (one DVE op per 512-slice).
  - V projection in natural [key, d] orientation (kin slices as lhsT).
  - Scores S^T[k, q] = K Q^T in fp16, 3-way row-packed (tile_position via
    base partitions 0/32/64), 3 k-tiles per PSUM group (3 banks), strict
    causal: diagonal sub-tiles skip their fully-masked column prefix.
  - Max-free softmax: one exp activation per group [128, 512*nsub] on the
    ACT engine; nothing else runs on the scalar queue. Strict-causal 0/1
    mask applied post-exp on GpSimd. Garbage columns (masked prefixes) are
    never read: PV streams rhs starting at the prefix offset.
  - PV 2-way column-packed (col groups at partitions 0 and 64, 33-wide
    V|ones weights); denominator via the ones column; groups merged in the
    tail with one DVE add.
  - Normalization without transposes: per-block column sums -> reshape DMA
    [8,64] -> reciprocal -> [1,512] -> K=1 replicate matmul -> tensor_mul.
    Output stays in O^T layout [32, 4096] per core.
"""

import math

import numpy as np

import concourse.bass as bass
import concourse.mybir as mybir
from concourse import bacc
from concourse.tile import TileContext
from concourse.bass_utils import run_bass_kernel_spmd

# Problem constants (hardcoded per harness contract).
B, CQ, CK, CH, NH, H, W = 1, 256, 256, 256, 8, 64, 64
DH = CH // NH            # 32
N = H * W                # 4096
QB = 512                 # queries per block
NQB = N // QB            # 8
KT = 128                 # keys per k-tile
NKT = N // KT            # 32
GS = 3                   # k-tiles per score group (3 PSUM banks, 3-way row pack)
SCALE = 1.0 / math.sqrt(DH)
NWARM = 10               # PE warmup matmuls (HAM clock-gate warmup)
HACK_A = 1477.3195 / math.sqrt(DH)  # fp16 Schraudolph slope (folds 1/sqrt(DH))
HACK_B = 15301.0                    # p-weighted-mean-zero intercept

F32 = mybir.dt.float32
F32R = mybir.dt.float32r
F16 = mybir.dt.float16
I16 = mybir.dt.int16

_CACHED_NC = None


def _build():
    nc = bacc.Bacc("TRN2", target_bir_lowering=False, debug=False, num_devices=1)

    qin_d = nc.dram_tensor("qin", [CQ, N], F32, kind="ExternalInput")
    kin_d = nc.dram_tensor("kin", [CK, N], F32, kind="ExternalInput")
    wq_d = nc.dram_tensor("wqt3", [CQ, 96], F32, kind="ExternalInput")
    wk_d = nc.dram_tensor("wkt3", [CK, 96], F32, kind="ExternalInput")
    wv_d = nc.dram_tensor("wvt", [CK, DH], F32, kind="ExternalInput")
    bq_d = nc.dram_tensor("bq3", [96, 1], F32, kind="ExternalInput")
    bk_d = nc.dram_tensor("bk3", [96, 1], F32, kind="ExternalInput")
    bv_d = nc.dram_tensor("bvr", [128, 4 * DH], F32, kind="ExternalInput")
    out_d = nc.dram_tensor("out", [DH, N], F32, kind="ExternalOutput")

    # Strict-causal within-tile mask: tm[kk, qq] = 1.0 iff kk < qq.
    tm_np = (np.arange(128)[:, None] < np.arange(128)[None, :]).astype(np.float16)
    tm_d = nc.inline_tensor(tm_np, name="tmask")
    ones_d = nc.inline_tensor(np.ones((1, DH), dtype=np.float32), name="onesd")
    eps_np = np.zeros((DH + 1, 1), dtype=np.float32)
    eps_np[DH, 0] = 1e-30
    eps_d = nc.inline_tensor(eps_np, name="epsd")
    sel_np = np.zeros((97, DH), dtype=np.float16)
    for _d in range(DH):
        sel_np[_d, _d] = 1.0
        sel_np[64 + _d, _d] = 1.0
    sel_d = nc.inline_tensor(sel_np, name="seld")
    dsel_np = np.zeros((97, 1), dtype=np.float16)
    dsel_np[DH, 0] = 1.0
    dsel_np[96, 0] = 1.0
    dsel_d = nc.inline_tensor(dsel_np, name="dseld")
    ones97_d = nc.inline_tensor(np.ones((1, 97), dtype=np.float32), name="ones97d")

    kin_ap = kin_d.ap().rearrange("(c p) n -> p c n", p=128).bitcast(F32R)
    qin_ap = qin_d.ap().rearrange("(c p) n -> p c n", p=128).bitcast(F32R)
    wk_ap = wk_d.ap().rearrange("(c p) m -> p c m", p=128).bitcast(F32R)
    wq_ap = wq_d.ap().rearrange("(c p) m -> p c m", p=128).bitcast(F32R)
    wv_ap = wv_d.ap().rearrange("(c p) m -> p c m", p=128).bitcast(F32R)

    with TileContext(nc) as tc:
        with (
            tc.tile_pool(name="constp", bufs=1) as constp,
            tc.tile_pool(name="bigp", bufs=1) as bigp,
            tc.tile_pool(name="workp", bufs=4) as workp,
            tc.tile_pool(name="spool", bufs=2, space="PSUM") as spool,
            tc.tile_pool(name="mpool", bufs=1, space="PSUM") as mpool,
        ):
            # ---- big SBUF tiles ----
            kin_sb = bigp.tile([128, 2, N], F32R, name="kin_sb")
            qin_sb = bigp.tile([128, 2, N], F32R, name="qin_sb")
            kT3 = bigp.tile([96, N], F16, name="kT3")    # 3x replicated k^T
            qT3 = bigp.tile([96, N], F16, name="qT3")    # 3x replicated q^T
            # v_all[kk, t, 0:DH] = v[128t+kk, :]; col DH is the ones column
            v_all = bigp.tile([128, NKT, 48], F16, name="v_all")
            warm = bigp.tile([32, 640], F16, name="warm")

            # ---- DMAs: weights + first slices first; kin on sync, qin on
            # gpsimd (cheap issue), nothing on the scalar queue ----
            wk_sb = constp.tile([128, 2, 96], F32R, name="wk_sb")
            nc.sync.dma_start(wk_sb[:], wk_ap)
            wq_sb = constp.tile([128, 2, 96], F32R, name="wq_sb")
            nc.sync.dma_start(wq_sb[:], wq_ap)
            slA = slice(0, QB)
            nc.sync.dma_start(kin_sb[:, :, slA], kin_ap[:, :, slA])
            nc.sync.dma_start(qin_sb[:, :, slA], qin_ap[:, :, slA])
            wv_sb = constp.tile([128, 2, DH], F32R, name="wv_sb")
            nc.gpsimd.dma_start(wv_sb[:], wv_ap)
            bq_sb = constp.tile([96, 1], F32, name="bq_sb")
            nc.gpsimd.dma_start(bq_sb[:], bq_d.ap())
            bk_sb = constp.tile([96, 1], F32, name="bk_sb")
            nc.gpsimd.dma_start(bk_sb[:], bk_d.ap())
            bv_sb = constp.tile([128, 4, DH], F32, name="bv_sb")
            nc.gpsimd.dma_start(bv_sb[:], bv_d.ap().rearrange("p (t d) -> p t d", t=4))
            tm_sb = constp.tile([128, 128], F16, name="tm_sb")
            nc.gpsimd.dma_start(tm_sb[:], tm_d.ap())
            ones_sb = constp.tile([1, DH], F32R, name="ones_sb")
            nc.gpsimd.dma_start(ones_sb[:], ones_d.ap().bitcast(F32R))
            eps_sb = constp.tile([DH + 1, 1], F32, name="eps_sb")
            nc.gpsimd.dma_start(eps_sb[:], eps_d.ap())
            sel_sb = constp.tile([97, DH], F16, name="sel_sb")
            nc.gpsimd.dma_start(sel_sb[:], sel_d.ap())
            dsel_sb = constp.tile([97, 1], F16, name="dsel_sb")
            nc.gpsimd.dma_start(dsel_sb[:], dsel_d.ap())
            ones97_sb = constp.tile([1, 97], F32R, name="ones97_sb")
            nc.gpsimd.dma_start(ones97_sb[:], ones97_d.ap().bitcast(F32R))
            nc.vector.memset(warm[:], 0.0)
            nc.vector.memset(v_all[:, :, DH : DH + 1], 1.0)
            for s in range(1, NQB):
                sl = slice(QB * s, QB * (s + 1))
                nc.gpsimd.dma_start(kin_sb[:, :, sl], kin_ap[:, :, sl])
                nc.gpsimd.dma_start(qin_sb[:, :, sl], qin_ap[:, :, sl])

            # ---- PE warmup: keep the array busy through the DMA window so
            # the HAM un-throttles before real matmuls arrive ----
            wm_ps = mpool.tile([128, 512], F32, name="wm_ps", tag="p")
            for i in range(NWARM):
                nc.tensor.matmul(
                    wm_ps[:], warm[0:32, 0:128], warm[0:32, 128:640],
                    start=(i == 0), stop=(i == NWARM - 1),
                )

            stage_q = []  # deferred tail stages, advanced one per score group

            def emit_kq(s):
                ksl = slice(QB * s, QB * (s + 1))
                pjk = mpool.tile([96, 512], F32, name="pjk", tag="p")
                for ch in range(2):
                    nc.tensor.matmul(
                        pjk[:], wk_sb[:, ch, :], kin_sb[:, ch, ksl],
                        start=(ch == 0), stop=(ch == 1),
                    )
                nc.vector.tensor_scalar_add(kT3[:, ksl], pjk[:], bk_sb[:])
                pjq = mpool.tile([96, 512], F32, name="pjq", tag="p")
                for ch in range(2):
                    nc.tensor.matmul(
                        pjq[:], wq_sb[:, ch, :], qin_sb[:, ch, ksl],
                        start=(ch == 0), stop=(ch == 1),
                    )
                nc.vector.tensor_scalar_add(qT3[:, ksl], pjq[:], bq_sb[:])

            def emit_v4(s):
                # 4 v-tiles of one slice into one PSUM tile: no per-tile DVE
                # round-trips on the single-bank "p" ring, one batched add
                pv4 = mpool.tile([128, 4, DH], F32, name="pv4", tag="p")
                for ti in range(4):
                    t = 4 * s + ti
                    nsl = slice(128 * t, 128 * (t + 1))
                    for ch in range(2):
                        nc.tensor.matmul(
                            pv4[:, ti, :], kin_sb[:, ch, nsl], wv_sb[:, ch, :],
                            start=(ch == 0), stop=(ch == 1),
                        )
                nc.vector.tensor_add(
                    v_all[:, 4 * s : 4 * s + 4, 0:DH], pv4[:], bv_sb[:]
                )

            def tail_b(st):
                cs8r = workp.tile([8, 64], F32, name="cs8r")
                nc.vector.reciprocal(cs8r[:], st["cs8"][:])
                csr = workp.tile([1, 512], F32R, name="csr")
                nc.sync.dma_start(csr[:], cs8r[:].bitcast(F32R))
                st.update(csr=csr)

            def tail_c0(st):
                qb = st["qb"]
                rep_ps = mpool.tile([DH, 512], F32, name="rep_ps", tag="p")
                nc.tensor.matmul(
                    rep_ps[:], ones_sb[:], st["csr"][:], start=True, stop=True
                )
                out_sb = workp.tile([DH, 512], F32, name="out_sb")
                nc.vector.tensor_mul(out_sb[:], st["o_sb"][:], rep_ps[:])
                nc.sync.dma_start(
                    out_d.ap()[:, QB * qb : QB * (qb + 1)], out_sb[:]
                )

            def tail_c1(st):
                rep_ps = mpool.tile([97, 512], F32, name="rep97_ps", tag="p")
                nc.tensor.matmul(
                    rep_ps[:], ones97_sb[:], st["csr"][:], start=True, stop=True
                )
                st.update(rep_ps=rep_ps)

            def tail_c2(st):
                o2n_sb = workp.tile([97, 512], F16, name="o2n_sb")
                nc.vector.tensor_mul(o2n_sb[:], st["o2_sb"][:], st["rep_ps"][:])
                st.update(o2n_sb=o2n_sb)

            def tail_c3(st):
                qb = st["qb"]
                out_ps = mpool.tile([DH, 512], F32, name="out_ps", tag="p")
                nc.tensor.matmul(
                    out_ps[:], sel_sb[:], st["o2n_sb"][:], start=True, stop=True
                )
                out_sb = workp.tile([DH, 512], F32, name="out_sb")
                nc.vector.tensor_copy(out_sb[:], out_ps[:])
                nc.sync.dma_start(
                    out_d.ap()[:, QB * qb : QB * (qb + 1)], out_sb[:]
                )

            pends = []  # ('pv', tiles, p_sb, o_ps, qb, nkt) | ('tail', qb, o_ps)

            def pump():
                if not pends:
                    return
                e = pends.pop(0)
                if e[0] == "pv":
                    _, tiles, p_sb, o_ps, qb, nkt = e
                    for (u, j) in tiles:
                        o = max(0, 128 * j - QB * qb)
                        nc.tensor.matmul(
                            o_ps[:, o:512],
                            v_all[:, j, 0 : DH + 1],
                            p_sb[:, 512 * u + o : 512 * (u + 1)],
                            start=(j == 0),
                            stop=(j == nkt - 1),
                            skip_group_check=True,
                        )
                else:
                    _, qb, o_ps = e
                    o33_sb = workp.tile([DH + 1, 512], F32, name="o33_sb")
                    nc.vector.tensor_scalar_add(o33_sb[:], o_ps[:], eps_sb[:])
                    cs8 = workp.tile([8, 64], F32, name="cs8")
                    nc.sync.dma_start(cs8[:], o33_sb[DH : DH + 1, :])
                    st = {"qb": qb, "o_sb": o33_sb[0:DH, :], "cs8": cs8}
                    stage_q.append(lambda st=st: tail_b(st))
                    stage_q.append(lambda: None)
                    stage_q.append(lambda st=st: tail_c0(st))

            def emit_qb(qb):
                nkt = 4 * (qb + 1)
                ngr = (nkt + GS - 1) // GS
                o_ps = mpool.tile([DH + 1, 512], F32, name="o_ps", tag="o", bufs=1)

                for g in range(ngr):
                    sz = min(GS, nkt - GS * g)
                    tiles = [(u, GS * g + u) for u in range(sz)]
                    s_ps = spool.tile([128, GS * 512], F32, name="s_ps", tag="s")
                    for (u, j) in tiles:
                        o = max(0, 128 * j - QB * qb)
                        nc.tensor.matmul(
                            s_ps[:, 512 * u + o : 512 * (u + 1)],
                            kT3[32 * u : 32 * u + 32, 128 * j : 128 * (j + 1)],
                            qT3[32 * u : 32 * u + 32, QB * qb + o : QB * (qb + 1)],
                            start=True, stop=True,
                        )
                    p_sb = workp.tile([128, GS * 512], F16, name="p_sb", bufs=8)
                    gidx = GIDX[0]
                    GIDX[0] += 1
                    if gidx >= 6 and (gidx - 6) % 4 == 0:
                        # exp on DVE via fp16 Schraudolph bit-hack:
                        # exp(s*SCALE) ~= bitcast_fp16(int16(s*(A*SCALE) + B))
                        with nc.allow_low_precision(reason="dve exp bit-hack"):
                            nc.vector.tensor_scalar(
                                p_sb[:, 0 : 512 * sz].bitcast(I16),
                                s_ps[:, 0 : 512 * sz],
                                HACK_A,
                                HACK_B,
                                op0=mybir.AluOpType.mult,
                                op1=mybir.AluOpType.add,
                            )
                    else:
                        nc.scalar.activation(
                            p_sb[:, 0 : 512 * sz],
                            s_ps[:, 0 : 512 * sz],
                            mybir.ActivationFunctionType.Exp,
                            scale=SCALE,
                        )
                    for (u, j) in tiles:
                        o = 128 * j - QB * qb
                        if o >= 0:  # strict-causal mask on the diagonal window
                            nc.gpsimd.tensor_mul(
                                p_sb[:, 512 * u + o : 512 * u + o + 128],
                                p_sb[:, 512 * u + o : 512 * u + o + 128],
                                tm_sb[:],
                            )
                    pends.append(("pv", tiles, p_sb, o_ps, qb, nkt))
                    while len(pends) > 3:
                        pump()
                    if stage_q:
                        stage_q.pop(0)()
                pends.append(("tail", qb, o_ps))

            GIDX = [0]
            emit_kq(0)
            emit_kq(1)
            emit_v4(0)
            for qb in range(NQB):
                emit_qb(qb)
                if qb + 2 < NQB:
                    emit_kq(qb + 2)
                if qb + 1 < NQB:
                    emit_v4(qb + 1)
            while pends:
                pump()
            while stage_q:
                stage_q.pop(0)()

    nc.finalize()
    return nc


def _get_nc():
    global _CACHED_NC
    if _CACHED_NC is None:
        _CACHED_NC = _build()
    return _CACHED_NC


def _prep_in_maps(inputs):
    f = lambda a: np.ascontiguousarray(np.asarray(a, dtype=np.float32))
    query = f(inputs["query"]).reshape(CQ, N)
    key_feat = f(inputs["key_feat"]).reshape(CK, N)

    def wnorm(v, g):
        v = f(v)
        g = f(g)
        return g[:, None] * v / np.linalg.norm(v, axis=1, keepdims=True)

    wq = wnorm(inputs["vq"], inputs["gq"])
    wk = wnorm(inputs["vk"], inputs["gk"])
    wv = wnorm(inputs["vv"], inputs["gv"])
    bq, bk, bv = f(inputs["bq"]), f(inputs["bk"]), f(inputs["bv"])

    in_maps = []
    for c in range(NH):
        rows = slice(DH * c, DH * (c + 1))
        in_maps.append(
            {
                "qin": query,
                "kin": key_feat,
                "wqt3": np.ascontiguousarray(np.tile(wq[rows].T, (1, 3))),
                "wkt3": np.ascontiguousarray(np.tile(wk[rows].T, (1, 3))),
                "wvt": np.ascontiguousarray(wv[rows].T),
                "bq3": np.ascontiguousarray(np.tile(bq[rows], 3)[:, None]),
                "bk3": np.ascontiguousarray(np.tile(bk[rows], 3)[:, None]),
                "bvr": np.ascontiguousarray(np.tile(bv[rows][None, :], (128, 4))),
            }
        )
    return in_maps


def _run(inputs, trace=False, **kwargs):
    nc = _get_nc()
    in_maps = _prep_in_maps(inputs)
    res = None
    for attempt in range(3):
        try:
            res = run_bass_kernel_spmd(
                nc, in_maps, core_ids=list(range(NH)), trace=trace, **kwargs
            )
            break
        except Exception:
            if attempt == 2:
                raise

    out = np.empty((B, CH, H, W), dtype=np.float32)
    for c in range(NH):
        oc = res.results[c]["out"]  # [DH, N] (O^T layout)
        out[0, DH * c : DH * (c + 1)] = oc.reshape(DH, H, W)
    return out, res


def kernel(**inputs) -> np.ndarray:
    out, _ = _run(inputs, trace=False)
    return out
